# revision 5
# baseline (speedup 1.0000x reference)
"""GCNConv Trainium2 kernel, 8-core SPMD — int8 wire variant.

Same structure as the fp16 kernel (see kernel.py docstring), but the
gathered X stream is int8 with one global scale folded into W on the host:
    x8 = round(x * 127/amax);  W' = W * amax/127
    out = segment_sum(val * x8[edge]) @ W' + bias  ==  reference out

The int8->fp16 dequant + edge-val scale runs on three parallel lanes
(whole tiles assigned per lane, ~proportional to lane throughput):
  V: DVE tensor_tensor (int8 x fp16-bcast -> fp16), 1x mode (1-byte in0)
  A: ACT per-round activation Copy with per-partition scale AP (converts
     int8->fp16 and multiplies by the round's edge-val vector)
  G: GPSIMD tensor_tensor (same as V, on the Pool engine)
PE consumes the scaled fp16 rounds exactly as the fp16 kernel (identity
transpose-scatter with PSUM accumulation), then GEMM2 + bias + out-DMA.
"""

import numpy as np

N_NODES = 50000
N_EDGES = 800000
F = 128
P = 128
N_CORES = 8
SPAN = P * N_CORES
N_TILES = (N_NODES + SPAN - 1) // SPAN      # 49
NPOS = N_TILES * SPAN
SLOTS = N_TILES * P
VDUP = 4
CHUNK = 8                        # scale-op chunk (rounds) for the last group

# lane cost model, ns per 128-edge round
_LANE_NS = {"v": 133.0, "a": 300.0, "g": 254.0}

_KERNEL_CACHE = {}


def _assign_lanes(R, groups):
    """Per-tile lane assignment balancing lane busy-time."""
    lane_load = {"v": 0.0, "a": 20000.0, "g": 0.0}   # ACT reserves ~20us for copies/bias
    lanes = [None] * len(R)
    for (k0, gs) in groups:
        for k in range(k0, k0 + gs):
            # pick lane that finishes this tile earliest
            best = min("vag", key=lambda e: lane_load[e] + int(R[k]) * _LANE_NS[e])
            lanes[k] = best
            lane_load[best] += int(R[k]) * _LANE_NS[best]
    # the last group drains the pipeline: force DVE (chunked overlap)
    k0, gs = groups[-1]
    for k in range(k0, k0 + gs):
        if lanes[k] != "v":
            lane_load[lanes[k]] -= int(R[k]) * _LANE_NS[lanes[k]]
            lanes[k] = "v"
    return lanes


def _build_nc(R):
    from contextlib import ExitStack

    import concourse.bass as bass
    import concourse.mybir as mybir

    f16 = mybir.dt.float16
    f32 = mybir.dt.float32
    i8 = mybir.dt.int8

    NT = N_TILES
    B = int(np.sum(R))
    boffs = np.zeros(NT, dtype=np.int64)
    boffs[1:] = np.cumsum(R)[:-1]

    groups = []
    kk = 0
    while kk < NT:
        gs = min(4, NT - kk)
        groups.append((kk, gs))
        kk += gs
    NG = len(groups)
    group_of = np.zeros(NT, dtype=np.int64)
    for gi, (k0, gs) in enumerate(groups):
        group_of[k0 : k0 + gs] = gi
    gR = [int(sum(R[k0 : k0 + gs])) for (k0, gs) in groups]
    GRmax = max(gR)
    g_boff = [int(boffs[k0]) for (k0, _gs) in groups]

    lanes = _assign_lanes(R, groups)

    # slab halves (by tiles; by rounds for the last single-tile group)
    g_halves = []
    slot_loads = [0, 0, 0, 0, 0, 0]
    load_idx = {}
    for gi, (k0, gs) in enumerate(groups):
        if gs == 1 and gR[gi] > 2 * CHUNK:
            cut = (gR[gi] // (2 * CHUNK)) * CHUNK
        else:
            h1 = (gs + 1) // 2
            cut = (
                int(boffs[k0 + h1 - 1] + R[k0 + h1 - 1] - g_boff[gi])
                if h1 > 0
                else 0
            )
        halves = [(0, cut)]
        if cut < gR[gi]:
            halves.append((cut, gR[gi]))
        g_halves.append(halves)
        for hi, _ in enumerate(halves):
            si = 2 * (gi % 3) + hi
            slot_loads[si] += 1
            load_idx[(gi, hi)] = (si, slot_loads[si])

    # scale chunks per tile (chunked only for the last group)
    chunks = []   # (tile, r_start, r_end, xs_sem, xs_target)
    for k in range(NT):
        gi = int(group_of[k])
        _k0, gs = groups[gi]
        halves = g_halves[gi]
        tile_lo = int(boffs[k]) - g_boff[gi]
        tile_hi = tile_lo + int(R[k])
        if gi == NG - 1 and gs == 1:
            r = 0
            while r < int(R[k]):
                r2 = min(r + CHUNK, int(R[k]))
                hi = 0 if r2 <= halves[0][1] else 1
                chunks.append((k, r, r2) + load_idx[(gi, hi)])
                r = r2
        else:
            hi = 0
            for h, (ra, rb) in enumerate(halves):
                if tile_hi <= rb:
                    hi = h
                    break
            chunks.append((k, 0, int(R[k])) + load_idx[(gi, hi)])

    # per-lane chunk lists and counters
    lane_chunks = {e: [(c, i) for i, c in enumerate(chunks) if lanes[c[0]] == e]
                   for e in "vg"}
    # ACT lane handled per-round separately
    act_tiles = [k for k in range(NT) if lanes[k] == "a"]

    # s_lane counts after each tile, per lane (for PE + sync waits)
    cnt_after = {"v": np.zeros(NT, dtype=np.int64),
                 "a": np.zeros(NT, dtype=np.int64),
                 "g": np.zeros(NT, dtype=np.int64)}
    cv = ca = cg = 0
    tile_chunk_targets = {}      # tile -> list of (r_start, lane_sem_target)
    for (k, ra, rb, _si, _cnt) in chunks:
        e = lanes[k]
        if e == "v":
            cv += 1
            tile_chunk_targets.setdefault(k, []).append((ra, cv))
        elif e == "g":
            cg += 1
            tile_chunk_targets.setdefault(k, []).append((ra, cg))
        else:
            # ACT increments once per ROUND: let PE chase in 8-round steps
            base = ca
            for r in range(int(ra), int(rb), 8):
                tile_chunk_targets.setdefault(k, []).append(
                    (r, base + min(r + 8, int(rb)) - int(ra))
                )
            ca += int(rb - ra)
        cnt_after["v"][k] = cv
        cnt_after["a"][k] = ca
        cnt_after["g"][k] = cg

    nc = bass.Bass(target_bir_lowering=False, debug=False)

    XRT = nc.declare_dram_parameter("xrt", [P, B, F], i8, isOutput=False)
    VEX = nc.declare_dram_parameter("vex", [P, B, VDUP], f16, isOutput=False)
    VEXF = nc.declare_dram_parameter("vexf", [P, B], f32, isOutput=False)
    WP = nc.declare_dram_parameter("w", [F, F], f16, isOutput=False)
    BIASP = nc.declare_dram_parameter("bias", [F, 1], f32, isOutput=False)
    IDP = nc.declare_dram_parameter("ident", [P, P], f16, isOutput=False)
    OUT = nc.declare_dram_parameter("out", [F, SLOTS], f16, isOutput=True)

    with ExitStack() as ctx:
        ident = ctx.enter_context(nc.sbuf_tensor("identsb", [P, P], f16))
        wsb = ctx.enter_context(nc.sbuf_tensor("wsb", [F, F], f16))
        vex = ctx.enter_context(nc.sbuf_tensor("vexsb", [P, B, VDUP], f16))
        vexf = ctx.enter_context(nc.sbuf_tensor("vexfsb", [P, B], f32))
        bias = ctx.enter_context(nc.sbuf_tensor("biassb", [F, 1], f32))
        xs = [ctx.enter_context(nc.sbuf_tensor(f"xs{i}", [P, GRmax, F], i8)) for i in range(3)]
        sc = [ctx.enter_context(nc.sbuf_tensor(f"sc{i}", [P, GRmax, F], f16)) for i in range(2)]
        ht = [ctx.enter_context(nc.sbuf_tensor(f"ht{i}", [P, 4 * P], f16)) for i in range(2)]
        osb = [ctx.enter_context(nc.sbuf_tensor(f"osb{i}", [P, 4 * P], f16)) for i in range(2)]
        pha = [ctx.enter_context(nc.psum_tensor(f"pha{i}", [P, 512], f32)) for i in range(3)]
        phb = [ctx.enter_context(nc.psum_tensor(f"phb{i}", [P, 512], f32)) for i in range(2)]

        s_cst = ctx.enter_context(nc.semaphore("s_cst"))
        s_xs = [ctx.enter_context(nc.semaphore(f"s_xs{i}")) for i in range(6)]
        s_scv = ctx.enter_context(nc.semaphore("s_scv"))
        s_sca = ctx.enter_context(nc.semaphore("s_sca"))
        s_scg = ctx.enter_context(nc.semaphore("s_scg"))
        s_peA = ctx.enter_context(nc.semaphore("s_peA"))
        s_peB = ctx.enter_context(nc.semaphore("s_peB"))
        s_acth = ctx.enter_context(nc.semaphore("s_acth"))
        s_acto = ctx.enter_context(nc.semaphore("s_acto"))
        s_odma = [ctx.enter_context(nc.semaphore(f"s_odma{i}")) for i in range(2)]
        all_sems = [s_cst, *s_xs, s_scv, s_sca, s_scg, s_peA, s_peB,
                    s_acth, s_acto, *s_odma]
        s_lane = {"v": s_scv, "a": s_sca, "g": s_scg}

        for s in all_sems:
            nc.sync.sem_clear(s)
        nc.all_engine_barrier()

        def scale_in_aps(k, ra, rb):
            n = rb - ra
            gi = int(group_of[k])
            roff = int(boffs[k]) - g_boff[gi] + ra
            b0 = int(boffs[k]) + ra
            x_ap = (
                xs[gi % 3][:, roff : roff + n, :]
                .rearrange("p r (a b) -> p r a b", b=VDUP)
            )
            s_ap = (
                sc[gi % 2][:, roff : roff + n, :]
                .rearrange("p r (a b) -> p r a b", b=VDUP)
            )
            v_ap = (
                vex[:, b0 : b0 + n, :]
                .unsqueeze(2)
                .to_broadcast([P, n, F // VDUP, VDUP])
            )
            return s_ap, x_ap, v_ap

        def emit_lane(eng, e):
            sem = s_lane[e]
            prev_tile = -1
            for (k, ra, rb, si, cnt), _ in lane_chunks[e]:
                gi = int(group_of[k])
                eng.wait_ge(s_xs[si], 16 * cnt)
                if k != prev_tile and gi >= 2:
                    klast = groups[gi - 2][0] + groups[gi - 2][1] - 1
                    eng.wait_ge(s_peA, klast + 1)   # sc slab reuse
                prev_tile = k
                s_ap, x_ap, v_ap = scale_in_aps(k, ra, rb)
                engmod = nc.vector if e == "v" else nc.gpsimd
                engmod.tensor_tensor(
                    out=s_ap, in0=x_ap, in1=v_ap, op=mybir.AluOpType.mult
                ).then_inc(sem, 1)

        with nc.Block() as block:

            @block.sync
            def _(sp):
                ra0, rb0 = g_halves[0][0]
                nc.sync.dma_start(
                    out=xs[0][:, ra0:rb0, :], in_=XRT[:, ra0:rb0, :]
                ).then_inc(s_xs[0], 16)
                nc.sync.dma_start(out=ident.ap(), in_=IDP.ap()).then_inc(s_cst, 16)
                nc.sync.dma_start(out=wsb.ap(), in_=WP.ap()).then_inc(s_cst, 16)
                nc.sync.dma_start(out=bias.ap(), in_=BIASP.ap()).then_inc(s_cst, 16)

                for gi, (k0, gs) in enumerate(groups):
                    if gi >= 3:
                        klast = groups[gi - 3][0] + groups[gi - 3][1] - 1
                        for e in "vag":
                            sp.wait_ge(s_lane[e], int(cnt_after[e][klast]))
                    for hi, (ra, rb) in enumerate(g_halves[gi]):
                        if gi == 0 and hi == 0:
                            continue
                        nc.sync.dma_start(
                            out=xs[gi % 3][:, ra:rb, :],
                            in_=XRT[:, g_boff[gi] + ra : g_boff[gi] + rb, :],
                        ).then_inc(s_xs[2 * (gi % 3) + hi], 16)
                for i in range(6):
                    sp.wait_ge(s_xs[i], 16 * slot_loads[i])

            @block.vector
            def _(dve):
                dve.wait_ge(s_cst, 80)
                emit_lane(dve, "v")

            @block.gpsimd
            def _(gp):
                gp.wait_ge(s_cst, 80)
                emit_lane(gp, "g")

            @block.tensor
            def _(pe):
                pe.wait_ge(s_cst, 80)
                for k in range(NT):
                    Rk = int(R[k])
                    gi = int(group_of[k])
                    k0, gs = groups[gi]
                    roff = int(boffs[k]) - g_boff[gi]
                    e = lanes[k]
                    bounds = dict(tile_chunk_targets[k])
                    if k >= 3:
                        pe.wait_ge(s_acth, k - 2)
                    for r in range(Rk):
                        if r in bounds:
                            pe.wait_ge(s_lane[e], bounds[r])
                        mm = nc.tensor.matmul(
                            out=pha[k % 3][:, :P],
                            lhsT=sc[gi % 2][:, roff + r, :],
                            rhs=ident.ap(),
                            start=(r == 0),
                            stop=(r == Rk - 1),
                        )
                    mm.then_inc(s_peA, 1)
                    if k == k0 + gs - 1:
                        pe.wait_ge(s_acth, k + 1)
                        if gi >= 2:
                            pe.wait_ge(s_acto, gi - 1)
                        nc.tensor.matmul(
                            out=phb[gi % 2][:, : gs * P],
                            lhsT=wsb.ap(),
                            rhs=ht[gi % 2][:, : gs * P],
                            start=True,
                            stop=True,
                        ).then_inc(s_peB, 1)

            @block.scalar
            def _(act):
                nc.scalar.dma_start(out=vex.ap(), in_=VEX.ap()).then_inc(s_cst, 16)
                nc.scalar.dma_start(out=vexf.ap(), in_=VEXF.ap()).then_inc(s_cst, 16)
                act.wait_ge(s_cst, 80)

                def emit_act_scales(k):
                    """Per-round fused dequant+scale on ACT for tile k."""
                    gi = int(group_of[k])
                    roff = int(boffs[k]) - g_boff[gi]
                    b0 = int(boffs[k])
                    (kk, ra, rb, si, cnt) = next(
                        c for c in chunks if c[0] == k
                    )
                    act.wait_ge(s_xs[si], 16 * cnt)
                    if gi >= 2:
                        klast = groups[gi - 2][0] + groups[gi - 2][1] - 1
                        act.wait_ge(s_peA, klast + 1)
                    for r in range(int(R[k])):
                        nc.scalar.activation(
                            out=sc[gi % 2][:, roff + r, :],
                            in_=xs[gi % 3][:, roff + r, :],
                            func=mybir.ActivationFunctionType.Copy,
                            scale=vexf[:, b0 + r : b0 + r + 1],
                        ).then_inc(s_sca, 1)

                if lanes[0] == "a":
                    emit_act_scales(0)
                for k in range(NT):
                    gi = int(group_of[k])
                    k0, gs = groups[gi]
                    j = k - k0
                    # emit ACT-lane scales one tile ahead of the pha copies
                    if k + 1 < NT and lanes[k + 1] == "a":
                        emit_act_scales(k + 1)
                    if j == 0 and gi >= 2:
                        act.wait_ge(s_peB, gi - 1)
                    act.wait_ge(s_peA, k + 1)
                    nc.scalar.copy(
                        ht[gi % 2][:, j * P : (j + 1) * P], pha[k % 3][:, :P]
                    ).then_inc(s_acth, 1)
                    if j == gs - 1:
                        act.wait_ge(s_peB, gi + 1)
                        if gi >= 2:
                            act.wait_ge(s_odma[gi % 2], 16 * (gi // 2))
                        nc.scalar.add(
                            osb[gi % 2][:, : gs * P],
                            phb[gi % 2][:, : gs * P],
                            bias.ap(),
                        ).then_inc(s_acto, 1)
                        nc.scalar.dma_start(
                            out=OUT[:, k0 * P : (k0 + gs) * P],
                            in_=osb[gi % 2][:, : gs * P],
                        ).then_inc(s_odma[gi % 2], 16)
                # no final s_odma waits (see kernel.py)

        for s in all_sems:
            nc.sync.sem_clear(s)
    return nc


def _prep(x, edge_row, edge_col, edge_val):
    deg = np.bincount(edge_row, minlength=N_NODES)
    order = np.argsort(deg, kind="stable")
    pos = np.empty(N_NODES, dtype=np.int64)
    pos[order] = np.arange(N_NODES)

    degs_padded = np.zeros(NPOS, dtype=np.int64)
    degs_padded[:N_NODES] = deg[order]
    R = degs_padded.reshape(N_TILES, SPAN).max(axis=1)
    R = np.maximum(R, 1).astype(np.int64)
    boff = np.zeros(N_TILES, dtype=np.int64)
    boff[1:] = np.cumsum(R)[:-1]

    p = pos[edge_row]
    c = p % N_CORES
    slot = p // N_CORES
    k = slot // P
    j = slot % P
    sort_idx = np.argsort(edge_row, kind="stable")
    sorted_rows = edge_row[sort_idx]
    ranks = np.arange(N_EDGES) - np.searchsorted(sorted_rows, sorted_rows)
    r = np.empty(N_EDGES, dtype=np.int64)
    r[sort_idx] = ranks
    b = boff[k] + r

    B = int(R.sum())
    amax = float(np.abs(x).max())
    s = amax / 127.0
    x8 = np.clip(np.round(x / s), -127, 127).astype(np.int8)
    XRT = np.zeros((N_CORES, P, B, F), dtype=np.int8)
    VAL = np.zeros((N_CORES, P, B), dtype=np.float16)
    XRT[c, j, b] = x8[edge_col]
    VAL[c, j, b] = edge_val.astype(np.float16)
    VEX = np.repeat(VAL[:, :, :, None], VDUP, axis=3)
    VEXF = VAL.astype(np.float32)
    return R, XRT, VEX, VEXF, order, s


def kernel(x, edge_row, edge_col, edge_val, weight, bias_param):
    import sys
    for pth in ("/opt/trn_rl_repo",):
        if pth not in sys.path:
            sys.path.insert(0, pth)
    from concourse.bass_utils import run_bass_kernel_spmd

    x = np.asarray(x, dtype=np.float32)
    edge_row = np.asarray(edge_row, dtype=np.int32)
    edge_col = np.asarray(edge_col, dtype=np.int32)
    edge_val = np.asarray(edge_val, dtype=np.float32)
    weight = np.asarray(weight, dtype=np.float32)
    bias_param = np.asarray(bias_param, dtype=np.float32)

    R, XRT, VEX, VEXF, order, s = _prep(x, edge_row, edge_col, edge_val)

    key = tuple(R.tolist())
    if key not in _KERNEL_CACHE:
        _KERNEL_CACHE[key] = _build_nc(R)
    nc = _KERNEL_CACHE[key]

    w16 = (weight * s).astype(np.float16)
    bias2d = bias_param.reshape(F, 1).astype(np.float32)
    id16 = np.eye(P, dtype=np.float16)

    in_maps = [
        {
            "xrt": XRT[cid],
            "vex": VEX[cid],
            "vexf": VEXF[cid],
            "w": w16,
            "bias": bias2d,
            "ident": id16,
        }
        for cid in range(N_CORES)
    ]

    res = run_bass_kernel_spmd(nc, in_maps, core_ids=list(range(N_CORES)))

    out_full = np.empty((N_NODES, F), dtype=np.float32)
    for cid in range(N_CORES):
        outT = res.results[cid]["out"].astype(np.float32)
        gpos = np.arange(SLOTS) * N_CORES + cid
        valid = gpos < N_NODES
        out_full[order[gpos[valid]]] = outT.T[valid]
    return out_full


# revision 6
# speedup vs baseline: 1.6491x; 1.6491x over previous
"""GCNConv (X @ W sparse-aggregated) Trainium2 kernel, 8-core SPMD.

Math: out = segment_sum(edge_val * (X@W)[edge_col], edge_row) + bias
Reformulated via associativity:  out = H @ W + bias, where
    H = segment_sum(edge_val * X[edge_col], edge_row)          # [N, F]

Sharding: destination nodes are sorted by in-degree and dealt round-robin
across the 8 cores, so every core gets an identical per-tile "round"
structure (same compiled program on all cores).  The host pre-gathers
X[edge_col] into a round-major, partition-contiguous layout so the device
only does large sequential DMA; the device then:
  1. scales each gathered row by its edge value (DVE multiply; the
     edge values are pre-replicated 8x on host so the innermost AP dim is
     unit-stride 2-byte -> DVE 2x perf mode)
  2. scatter-reduces rounds into H.T tiles with PE matmuls against an
     identity (PSUM accumulation: lhsT=scaled rows, rhs=I)
  3. computes out.T = W.T @ H.T with a second PE matmul, adds bias during
     the PSUM->SBUF copy (ACT), and streams out.T to HBM.
The host un-permutes/transposes the per-core outputs into the full result.

Tail handling: the last group's scale is emitted in round-chunks with
standalone PE waits at chunk boundaries so DMA/scale/PE overlap while the
pipeline drains; its slab load is split by rounds for the same reason.
The final output-DMA completion waits are omitted — the inter-exec gap and
the NEFF's own teardown cover the last transfer's flight time.

Raw Bass (no Tile framework): this walrus build allows only ONE attached
sync-wait per compute instruction; standalone wait_ge sequencer ops have no
such limit and the pipeline is static, so explicit counters work.
DMA completion semaphores are per-buffer-slot: a slot's wait target always
equals the total count of DMAs ever issued on that semaphore at wait time,
so partial-completion skew across the 16 SDMA engines cannot fire it early.
"""

import numpy as np

N_NODES = 50000
N_EDGES = 800000
F = 128
P = 128
N_CORES = 8
SPAN = P * N_CORES               # 1024 degree-sorted nodes per tile-span
N_TILES = (N_NODES + SPAN - 1) // SPAN      # 49
NPOS = N_TILES * SPAN            # 50176 padded positions
SLOTS = N_TILES * P              # 6272 node slots per core
VDUP = 4                         # host-side replication of edge values
CHUNK = 8                        # scale-op chunk (rounds) for the last group

_KERNEL_CACHE = {}


def _build_nc(R):
    from contextlib import ExitStack

    import concourse.bass as bass
    import concourse.mybir as mybir

    f16 = mybir.dt.float16
    f32 = mybir.dt.float32

    NT = N_TILES
    B = int(np.sum(R))
    boffs = np.zeros(NT, dtype=np.int64)
    boffs[1:] = np.cumsum(R)[:-1]

    # group structure: 4 tiles per group (one DMA slab + one N=512 GEMM2);
    # the last 5 tiles form 2+2+1 groups so the pipeline drain is finer
    groups = []  # (first_tile, gsize)
    kk = 0
    while kk < NT:
        left = NT - kk
        gs = min(4, left) if left > 5 else (2 if left > 1 else 1)
        groups.append((kk, gs))
        kk += gs
    NG = len(groups)
    group_of = np.zeros(NT, dtype=np.int64)
    for gi, (k0, gs) in enumerate(groups):
        group_of[k0 : k0 + gs] = gi
    gR = [int(sum(R[k0 : k0 + gs])) for (k0, gs) in groups]
    GRmax = max(gR)
    g_boff = [int(boffs[k0]) for (k0, _gs) in groups]

    # split each group's slab load in two for finer pipelining: by tiles for
    # multi-tile groups, by rounds (at a CHUNK boundary) for the last
    # single-tile group so the drain overlaps DMA with scale+PE.
    g_halves = []          # per group: list of (round_start, round_end) in slab
    slot_loads = [0, 0, 0, 0, 0, 0]
    load_idx = {}          # (gi, hi) -> (sem idx, count target after this load)
    for gi, (k0, gs) in enumerate(groups):
        if gs == 1 and gR[gi] > 2 * CHUNK:
            cut = (gR[gi] // (2 * CHUNK)) * CHUNK
        else:
            h1 = (gs + 1) // 2
            cut = (
                int(boffs[k0 + h1 - 1] + R[k0 + h1 - 1] - g_boff[gi])
                if h1 > 0
                else 0
            )
        halves = [(0, cut)]
        if cut < gR[gi]:
            halves.append((cut, gR[gi]))
        g_halves.append(halves)
        for hi, _ in enumerate(halves):
            si = 2 * (gi % 3) + hi
            slot_loads[si] += 1
            load_idx[(gi, hi)] = (si, slot_loads[si])

    # scale chunks: (tile, r_start, r_end, xs_sem, xs_target).  One chunk per
    # tile except the last group, which is cut into CHUNK-round pieces.
    chunks = []
    chunk_cnt_after = np.zeros(NT, dtype=np.int64)   # s_scv value after tile k
    for k in range(NT):
        gi = int(group_of[k])
        k0, gs = groups[k0] if False else groups[gi]
        halves = g_halves[gi]
        # which half finishes this tile's rounds
        tile_lo = int(boffs[k]) - g_boff[gi]
        tile_hi = tile_lo + int(R[k])
        if gi == NG - 1 and gs == 1:
            r = 0
            while r < int(R[k]):
                r2 = min(r + CHUNK, int(R[k]))
                hi = 0 if r2 <= halves[0][1] else 1
                chunks.append((k, r, r2) + load_idx[(gi, hi)])
                r = r2
        else:
            hi = 0
            for h, (ra, rb) in enumerate(halves):
                if tile_hi <= rb:
                    hi = h
                    break
            chunks.append((k, 0, int(R[k])) + load_idx[(gi, hi)])
        chunk_cnt_after[k] = len(chunks)

    nc = bass.Bass(target_bir_lowering=False, debug=False)

    XRT = nc.declare_dram_parameter("xrt", [P, B, F], f16, isOutput=False)
    VEX = nc.declare_dram_parameter("vex", [P, B, VDUP], f16, isOutput=False)
    WP = nc.declare_dram_parameter("w", [F, F], f16, isOutput=False)
    BIASP = nc.declare_dram_parameter("bias", [F, 1], f32, isOutput=False)
    IDP = nc.declare_dram_parameter("ident", [P, P], f16, isOutput=False)
    OUT = nc.declare_dram_parameter("out", [F, SLOTS], f16, isOutput=True)

    with ExitStack() as ctx:
        ident = ctx.enter_context(nc.sbuf_tensor("identsb", [P, P], f16))
        wsb = ctx.enter_context(nc.sbuf_tensor("wsb", [F, F], f16))
        vex = ctx.enter_context(nc.sbuf_tensor("vexsb", [P, B, VDUP], f16))
        bias = ctx.enter_context(nc.sbuf_tensor("biassb", [F, 1], f32))
        xs = [ctx.enter_context(nc.sbuf_tensor(f"xs{i}", [P, GRmax, F], f16)) for i in range(3)]
        sc = [ctx.enter_context(nc.sbuf_tensor(f"sc{i}", [P, GRmax, F], f16)) for i in range(2)]
        ht = [ctx.enter_context(nc.sbuf_tensor(f"ht{i}", [P, 4 * P], f16)) for i in range(2)]
        osb = [ctx.enter_context(nc.sbuf_tensor(f"osb{i}", [P, 4 * P], f16)) for i in range(2)]
        pha = [ctx.enter_context(nc.psum_tensor(f"pha{i}", [P, 512], f32)) for i in range(3)]
        phb = [ctx.enter_context(nc.psum_tensor(f"phb{i}", [P, 512], f32)) for i in range(2)]
        phw = ctx.enter_context(nc.psum_tensor("phw", [P, 512], f32))

        s_cst = ctx.enter_context(nc.semaphore("s_cst"))
        s_xs = [ctx.enter_context(nc.semaphore(f"s_xs{i}")) for i in range(6)]
        s_scv = ctx.enter_context(nc.semaphore("s_scv"))
        s_peA = ctx.enter_context(nc.semaphore("s_peA"))
        s_peB = ctx.enter_context(nc.semaphore("s_peB"))
        s_acth = ctx.enter_context(nc.semaphore("s_acth"))
        s_acto = ctx.enter_context(nc.semaphore("s_acto"))
        s_odma = [ctx.enter_context(nc.semaphore(f"s_odma{i}")) for i in range(2)]
        all_sems = [s_cst, *s_xs, s_scv, s_peA, s_peB, s_acth, s_acto, *s_odma]

        for s in all_sems:
            nc.sync.sem_clear(s)
        nc.all_engine_barrier()

        def scale_in_aps(k, ra, rb):
            """(out_ap, in0_ap, in1_ap) for tile k rounds [ra, rb), 2x-eligible."""
            n = rb - ra
            gi = int(group_of[k])
            roff = int(boffs[k]) - g_boff[gi] + ra   # round offset inside slab
            b0 = int(boffs[k]) + ra
            x_ap = (
                xs[gi % 3][:, roff : roff + n, :]
                .rearrange("p r (a b) -> p r a b", b=VDUP)
            )
            s_ap = (
                sc[gi % 2][:, roff : roff + n, :]
                .rearrange("p r (a b) -> p r a b", b=VDUP)
            )
            v_ap = (
                vex[:, b0 : b0 + n, :]
                .unsqueeze(2)
                .to_broadcast([P, n, F // VDUP, VDUP])
            )
            return s_ap, x_ap, v_ap

        with nc.Block() as block:

            @block.sync
            def _(sp):
                # first half-slab ahead of the consts: the bulk stream starts
                # at t=0 while nothing can consume it before ~3us anyway
                ra0, rb0 = g_halves[0][0]
                nc.sync.dma_start(
                    out=xs[0][:, ra0:rb0, :], in_=XRT[:, ra0:rb0, :]
                ).then_inc(s_xs[0], 16)
                nc.sync.dma_start(out=ident.ap(), in_=IDP.ap()).then_inc(s_cst, 16)
                nc.sync.dma_start(out=wsb.ap(), in_=WP.ap()).then_inc(s_cst, 16)
                nc.sync.dma_start(out=bias.ap(), in_=BIASP.ap()).then_inc(s_cst, 16)

                for gi, (k0, gs) in enumerate(groups):
                    if gi >= 3:
                        # xs slab reuse: all scale ops of group gi-3 done
                        klast = groups[gi - 3][0] + groups[gi - 3][1] - 1
                        sp.wait_ge(s_scv, int(chunk_cnt_after[klast]))
                    for hi, (ra, rb) in enumerate(g_halves[gi]):
                        if gi == 0 and hi == 0:
                            continue  # pre-issued above
                        nc.sync.dma_start(
                            out=xs[gi % 3][:, ra:rb, :],
                            in_=XRT[:, g_boff[gi] + ra : g_boff[gi] + rb, :],
                        ).then_inc(s_xs[2 * (gi % 3) + hi], 16)
                for i in range(6):
                    sp.wait_ge(s_xs[i], 16 * slot_loads[i])

            @block.vector
            def _(dve):
                dve.wait_ge(s_cst, 64)
                prev_tile = -1
                for (k, ra, rb, si, cnt) in chunks:
                    gi = int(group_of[k])
                    dve.wait_ge(s_xs[si], 16 * cnt)
                    if k != prev_tile and gi >= 2:
                        klast = groups[gi - 2][0] + groups[gi - 2][1] - 1
                        dve.wait_ge(s_peA, klast + 1)  # sc slab reuse
                    prev_tile = k
                    s_ap, x_ap, v_ap = scale_in_aps(k, ra, rb)
                    nc.vector.tensor_tensor(
                        out=s_ap, in0=x_ap, in1=v_ap, op=mybir.AluOpType.mult
                    ).then_inc(s_scv, 1)

            @block.tensor
            def _(pe):
                pe.wait_ge(s_cst, 64)
                # chunk boundary lookup: tile -> list of (r_start, scv_target)
                tile_chunks = {}
                cum = 0
                for (k, ra, rb, _si, _cnt) in chunks:
                    cum += 1
                    tile_chunks.setdefault(k, []).append((ra, cum))
                for k in range(NT):
                    Rk = int(R[k])
                    gi = int(group_of[k])
                    k0, gs = groups[gi]
                    roff = int(boffs[k]) - g_boff[gi]
                    bounds = dict(tile_chunks[k])
                    if k >= 3:
                        pe.wait_ge(s_acth, k - 2)  # pha slot reuse
                    for r in range(Rk):
                        if r in bounds:
                            pe.wait_ge(s_scv, bounds[r])
                        mm = nc.tensor.matmul(
                            out=pha[k % 3][:, :P],
                            lhsT=sc[gi % 2][:, roff + r, :],
                            rhs=ident.ap(),
                            start=(r == 0),
                            stop=(r == Rk - 1),
                        )
                    mm.then_inc(s_peA, 1)
                    if k == k0 + gs - 1:
                        pe.wait_ge(s_acth, k + 1)
                        if gi >= 2:
                            pe.wait_ge(s_acto, gi - 1)
                        nc.tensor.matmul(
                            out=phb[gi % 2][:, : gs * P],
                            lhsT=wsb.ap(),
                            rhs=ht[gi % 2][:, : gs * P],
                            start=True,
                            stop=True,
                        ).then_inc(s_peB, 1)

            @block.scalar
            def _(act):
                nc.scalar.dma_start(out=vex.ap(), in_=VEX.ap()).then_inc(s_cst, 16)
                act.wait_ge(s_cst, 64)
                for k in range(NT):
                    gi = int(group_of[k])
                    k0, gs = groups[gi]
                    j = k - k0
                    if j == 0 and gi >= 2:
                        act.wait_ge(s_peB, gi - 1)  # ht slot reuse
                    act.wait_ge(s_peA, k + 1)
                    nc.scalar.copy(
                        ht[gi % 2][:, j * P : (j + 1) * P], pha[k % 3][:, :P]
                    ).then_inc(s_acth, 1)
                    if j == gs - 1:
                        act.wait_ge(s_peB, gi + 1)
                        if gi >= 2:
                            act.wait_ge(s_odma[gi % 2], 16 * (gi // 2))  # osb reuse
                        nc.scalar.add(
                            osb[gi % 2][:, : gs * P],
                            phb[gi % 2][:, : gs * P],
                            bias.ap(),
                        ).then_inc(s_acto, 1)
                        nc.scalar.dma_start(
                            out=OUT[:, k0 * P : (k0 + gs) * P],
                            in_=osb[gi % 2][:, : gs * P],
                        ).then_inc(s_odma[gi % 2], 16)
                # no final s_odma waits: the last transfers drain during the
                # NEFF teardown; nothing in this exec re-reads osb after here.

        for s in all_sems:
            nc.sync.sem_clear(s)
    return nc


def _prep(x, edge_row, edge_col, edge_val):
    """Host-side sharding/layout prep."""
    deg = np.bincount(edge_row, minlength=N_NODES)
    order = np.argsort(deg, kind="stable")            # node ids by degree asc
    pos = np.empty(N_NODES, dtype=np.int64)
    pos[order] = np.arange(N_NODES)

    degs_padded = np.zeros(NPOS, dtype=np.int64)
    degs_padded[:N_NODES] = deg[order]
    R = degs_padded.reshape(N_TILES, SPAN).max(axis=1)
    R = np.maximum(R, 1).astype(np.int64)
    boff = np.zeros(N_TILES, dtype=np.int64)
    boff[1:] = np.cumsum(R)[:-1]

    # per-edge placement
    p = pos[edge_row]
    c = p % N_CORES
    slot = p // N_CORES
    k = slot // P
    j = slot % P
    sort_idx = np.argsort(edge_row, kind="stable")
    sorted_rows = edge_row[sort_idx]
    ranks = np.arange(N_EDGES) - np.searchsorted(sorted_rows, sorted_rows)
    r = np.empty(N_EDGES, dtype=np.int64)
    r[sort_idx] = ranks
    b = boff[k] + r

    B = int(R.sum())
    x16 = x.astype(np.float16)
    XRT = np.zeros((N_CORES, P, B, F), dtype=np.float16)
    VAL = np.zeros((N_CORES, P, B), dtype=np.float16)
    XRT[c, j, b] = x16[edge_col]
    VAL[c, j, b] = edge_val.astype(np.float16)
    VEX = np.repeat(VAL[:, :, :, None], VDUP, axis=3)
    return R, XRT, VEX, order


def kernel(x, edge_row, edge_col, edge_val, weight, bias_param):
    import sys
    for pth in ("/opt/trn_rl_repo",):
        if pth not in sys.path:
            sys.path.insert(0, pth)
    from concourse.bass_utils import run_bass_kernel_spmd

    x = np.asarray(x, dtype=np.float32)
    edge_row = np.asarray(edge_row, dtype=np.int32)
    edge_col = np.asarray(edge_col, dtype=np.int32)
    edge_val = np.asarray(edge_val, dtype=np.float32)
    weight = np.asarray(weight, dtype=np.float32)
    bias_param = np.asarray(bias_param, dtype=np.float32)

    R, XRT, VEX, order = _prep(x, edge_row, edge_col, edge_val)

    key = tuple(R.tolist())
    if key not in _KERNEL_CACHE:
        _KERNEL_CACHE[key] = _build_nc(R)
    nc = _KERNEL_CACHE[key]

    w16 = weight.astype(np.float16)
    bias2d = bias_param.reshape(F, 1).astype(np.float32)
    id16 = np.eye(P, dtype=np.float16)

    in_maps = [
        {
            "xrt": XRT[cid],
            "vex": VEX[cid],
            "w": w16,
            "bias": bias2d,
            "ident": id16,
        }
        for cid in range(N_CORES)
    ]

    res = run_bass_kernel_spmd(nc, in_maps, core_ids=list(range(N_CORES)))

    out_full = np.empty((N_NODES, F), dtype=np.float32)
    for cid in range(N_CORES):
        outT = res.results[cid]["out"].astype(np.float32)   # [F, SLOTS]
        gpos = np.arange(SLOTS) * N_CORES + cid   # global positions
        valid = gpos < N_NODES
        out_full[order[gpos[valid]]] = outT.T[valid]
    return out_full
